# revision 10
# baseline (speedup 1.0000x reference)
"""Trainium2 Bass kernel for nn_CrossMarketCompoundEmbedding.

Output[i] = concat(price_w[0], size_w[0], exchange_w[i%3], pair_w[i%4])
for i in [0, 65536) -> [65536, 512] f32.

The output is periodic with period lcm(3,4)=12 rows (one "super-row" of
12*512 f32 = 24 KiB). Per core (8 cores, 8192 rows each = 16 MiB) the
kernel is pure HBM-write bandwidth: stage one super-row per SBUF
partition (all 128 partitions identical content, phase-shifted per core
on the host), then blast it to the output DRAM with a few large DMAs.

Layout: SBUF tile [128, 6144] f32, partition p supplies output rows
[c*1536 + p*12, c*1536 + p*12 + 12) of each 1536-row chunk. Since
1536 % 12 == 0 the same tile serves every chunk.
"""

import numpy as np

EMBED_DIM = 512
D4 = EMBED_DIM // 4
NUM_FEATURES = 65536
N_CORES = 8
ROWS_PER_CORE = NUM_FEATURES // N_CORES  # 8192
PERIOD = 12                              # lcm(3, 4)
SUPER = PERIOD * EMBED_DIM               # 6144 f32 per partition
CHUNK_ROWS = 128 * PERIOD                # 1536 rows per full-tile store
N_FULL = ROWS_PER_CORE // CHUNK_ROWS     # 5 full chunks -> 7680 rows
REM_ROWS = ROWS_PER_CORE - N_FULL * CHUNK_ROWS   # 512
REM_PARTS = REM_ROWS // PERIOD           # 42 partitions -> 504 rows
TAIL_ROWS = REM_ROWS - REM_PARTS * PERIOD  # 8 rows

_CACHE = {}

# test.py hooks (harness ignores these)
TRACE = False
LAST_EXEC_NS = None
LAST_RESULTS = None


def _build_program():
    import concourse.bass as bass
    import concourse.bacc as bacc
    import concourse.mybir as mybir

    nc = bacc.Bacc(
        "TRN2",
        target_bir_lowering=False,
        debug=False,
        enable_asserts=False,
        num_devices=N_CORES,
    )
    f32 = mybir.dt.float32
    block = nc.dram_tensor("block", [128, SUPER], f32, kind="ExternalInput").ap()
    out = nc.dram_tensor("out", [ROWS_PER_CORE, EMBED_DIM], f32, kind="ExternalOutput").ap()

    NPIECE = 4
    PW = SUPER // NPIECE  # 1536 cols per load piece
    # Chunk plan: engines 13/15 run ~20% slower than the rest, so two of
    # the five big chunk stores use only partitions 0-119 (dropping those
    # engines' partitions 120-127) to rebalance per-engine bytes.
    # Rows: c0..c2 are 128x12=1536 rows, c3/c4 are 120x12=1440 rows.
    CH = [(0, 1536, 128), (1536, 1536, 128), (3072, 1536, 128),
          (4608, 1440, 120), (6048, 1440, 120)]
    REM_START = 7488
    REM_P = 58  # 58 partitions (stride 2) x 12 rows = 696 rows
    N_STORES = NPIECE + 4 + 2  # c0 pieces + c1..c4 + rem + tail

    def chunk_ap(start, rows, parts):
        return out[start : start + rows].rearrange("(p r) d -> p (r d)", r=PERIOD)

    with (
        nc.sbuf_tensor("pat", [128, SUPER], f32) as t,
        nc.semaphore("ld_sem0") as ld0,
        nc.semaphore("ld_sem1") as ld1,
        nc.semaphore("ld_sem2") as ld2,
        nc.semaphore("ld_sem3") as ld3,
        nc.semaphore("st_sem") as st_sem,
        nc.Block() as blk,
    ):
        ld_sems = [ld0, ld1, ld2, ld3]
        rem = out[REM_START : REM_START + REM_P * PERIOD]
        rem = rem.rearrange("(p r) d -> p (r d)", r=PERIOD)  # [58, SUPER]
        tail = out[ROWS_PER_CORE - TAIL_ROWS :].rearrange(
            "(p r) d -> p (r d)", p=1
        )  # [1, TAIL_ROWS*512]
        c0 = chunk_ap(*CH[0])

        # SP ring: the four load pieces, then full-tile stores.
        # ACT ring: chunk-0 stores chase the load pieces (cross-ring, so
        # they drain concurrently with later loads), then more full stores.
        @blk.sync
        def _(sync):
            for i in range(NPIECE):
                sync.dma_start(
                    t[:, i * PW : (i + 1) * PW], block[:, i * PW : (i + 1) * PW]
                ).then_inc(ld_sems[i], 16)
            for s in ld_sems:
                sync.wait_ge(s, 16)
            for k in (1, 3):
                start, rows, parts = CH[k]
                sync.dma_start(chunk_ap(*CH[k]), t[:parts, :]).then_inc(st_sem, 16)
            sync.dma_start(rem, t[0 : 2 * REM_P : 2, :]).then_inc(st_sem, 16)
            sync.dma_start(tail, t[:1, : TAIL_ROWS * EMBED_DIM]).then_inc(st_sem, 16)
            sync.wait_ge(st_sem, 16 * N_STORES)

        @blk.scalar
        def _(scalar):
            for i in range(NPIECE):
                scalar.wait_ge(ld_sems[i], 16)
                scalar.dma_start(
                    c0[:, i * PW : (i + 1) * PW], t[:, i * PW : (i + 1) * PW]
                ).then_inc(st_sem, 16)
            for k in (2, 4):
                start, rows, parts = CH[k]
                scalar.dma_start(chunk_ap(*CH[k]), t[:parts, :]).then_inc(st_sem, 16)
    nc.compile()
    return nc


def _get_program():
    if "nc" not in _CACHE:
        _CACHE["nc"] = _build_program()
    return _CACHE["nc"]


def _host_blocks(price_w, size_w, exchange_w, pair_w):
    """Per-core [128, SUPER] f32 pattern blocks (all partitions identical)."""
    idx = np.arange(PERIOD)
    row12 = np.concatenate(
        [
            np.broadcast_to(price_w[0], (PERIOD, D4)),
            np.broadcast_to(size_w[0], (PERIOD, D4)),
            exchange_w[idx % 3],
            pair_w[idx % 4],
        ],
        axis=-1,
    ).astype(np.float32)  # [12, 512]
    blocks = []
    for c in range(N_CORES):
        base = c * ROWS_PER_CORE
        s = row12[(base + idx) % PERIOD].reshape(-1)  # [SUPER]
        blocks.append(np.ascontiguousarray(np.broadcast_to(s, (128, SUPER))))
    return blocks


def kernel(num_features, price_w, size_w, exchange_w, pair_w):
    global LAST_EXEC_NS, LAST_RESULTS
    from concourse.bass_utils import run_bass_kernel_spmd

    assert int(num_features) == NUM_FEATURES
    price_w = np.asarray(price_w, dtype=np.float32)
    size_w = np.asarray(size_w, dtype=np.float32)
    exchange_w = np.asarray(exchange_w, dtype=np.float32)
    pair_w = np.asarray(pair_w, dtype=np.float32)

    nc = _get_program()
    in_maps = [{"block": b} for b in _host_blocks(price_w, size_w, exchange_w, pair_w)]
    res = run_bass_kernel_spmd(nc, in_maps, list(range(N_CORES)), trace=TRACE)
    LAST_EXEC_NS = res.exec_time_ns
    LAST_RESULTS = res
    return np.concatenate([res.results[c]["out"] for c in range(N_CORES)], axis=0)


# revision 12
# speedup vs baseline: 1.4536x; 1.4536x over previous
"""Trainium2 Bass kernel for nn_CrossMarketCompoundEmbedding.

Output[i] = concat(price_w[0], size_w[0], exchange_w[i%3], pair_w[i%4])
for i in [0, 65536) -> [65536, 512] f32.

The output is periodic with period lcm(3,4)=12 rows (one "super-row" of
12*512 f32 = 24 KiB). Per core (8 cores, 8192 rows each = 16 MiB) the
kernel is pure HBM-write bandwidth: stage one super-row per SBUF
partition (all 128 partitions identical content, phase-shifted per core
on the host), then blast it to the output DRAM with a few large DMAs.

Layout: SBUF tile [128, 6144] f32, partition p supplies output rows
[c*1536 + p*12, c*1536 + p*12 + 12) of each 1536-row chunk. Since
1536 % 12 == 0 the same tile serves every chunk.
"""

import numpy as np

EMBED_DIM = 512
D4 = EMBED_DIM // 4
NUM_FEATURES = 65536
N_CORES = 8
ROWS_PER_CORE = NUM_FEATURES // N_CORES  # 8192
PERIOD = 12                              # lcm(3, 4)
SUPER = PERIOD * EMBED_DIM               # 6144 f32 per partition
CHUNK_ROWS = 128 * PERIOD                # 1536 rows per full-tile store
N_FULL = ROWS_PER_CORE // CHUNK_ROWS     # 5 full chunks -> 7680 rows
REM_ROWS = ROWS_PER_CORE - N_FULL * CHUNK_ROWS   # 512
REM_PARTS = REM_ROWS // PERIOD           # 42 partitions -> 504 rows
TAIL_ROWS = REM_ROWS - REM_PARTS * PERIOD  # 8 rows

_CACHE = {}

# test.py hooks (harness ignores these)
TRACE = False
LAST_EXEC_NS = None
LAST_RESULTS = None


def _build_program():
    import concourse.bass as bass
    import concourse.bass as bass
    import concourse.bacc as bacc
    import concourse.mybir as mybir

    nc = bacc.Bacc(
        "TRN2",
        target_bir_lowering=False,
        debug=False,
        enable_asserts=False,
        num_devices=N_CORES,
    )
    f32 = mybir.dt.float32
    block = nc.dram_tensor("block", [128, SUPER], f32, kind="ExternalInput").ap()
    out = nc.dram_tensor("out", [ROWS_PER_CORE, EMBED_DIM], f32, kind="ExternalOutput").ap()

    NPIECE = 4
    PW = SUPER // NPIECE  # 1536 cols per load piece
    N_STORES = NPIECE + 4 + 4 + 1  # c0 pieces + c1..c4 + 4 rem strips + tail
    REM_START = N_FULL * CHUNK_ROWS      # 7680
    REM_BANDS = [0, 32, 64, 86]          # SBUF partition band per rem strip

    with (
        nc.sbuf_tensor("pat", [128, SUPER], f32) as t,
        nc.semaphore("ld_sem0") as ld0,
        nc.semaphore("ld_sem1") as ld1,
        nc.semaphore("ld_sem2") as ld2,
        nc.semaphore("ld_sem3") as ld3,
        nc.semaphore("st_sem") as st_sem,
        nc.Block() as blk,
    ):
        ld_sems = [ld0, ld1, ld2, ld3]

        def chunk(k):  # [128, SUPER] view of chunk k's rows
            return out[k * CHUNK_ROWS : (k + 1) * CHUNK_ROWS].rearrange(
                "(p r) d -> p (r d)", r=PERIOD
            )

        # Remainder rows 7680..8184 as 4 strips: strip j writes rows
        # 7680+12k+3j..+2 (k<42) from tile cols [1536j, 1536j+1536), each
        # on a different partition band so descriptors spread over engines.
        def rem_strip(j):
            dst = out[REM_START + 3 * j :]
            dst = bass.AP(dst.tensor, dst.offset, [[PERIOD * EMBED_DIM, REM_PARTS], [1, 3 * EMBED_DIM]])
            b = REM_BANDS[j]
            src = t[b : b + REM_PARTS, j * PW : (j + 1) * PW]
            return dst, src

        tail = out[ROWS_PER_CORE - TAIL_ROWS :].rearrange(
            "(p r) d -> p (r d)", p=1
        )  # [1, TAIL_ROWS*512]
        c0 = chunk(0)

        # SP ring: the four load pieces, then full-tile stores.
        # ACT ring: chunk-0 stores chase the load pieces (cross-ring so they
        # drain while later loads still stream), then more full stores.
        @blk.sync
        def _(sync):
            for i in range(NPIECE):
                sync.dma_start(
                    t[:, i * PW : (i + 1) * PW], block[:, i * PW : (i + 1) * PW]
                ).then_inc(ld_sems[i], 16)
            for s in ld_sems:
                sync.wait_ge(s, 16)
            sync.dma_start(chunk(1), t[:, :]).then_inc(st_sem, 16)
            for j in (0, 2):
                d, s = rem_strip(j)
                sync.dma_start(d, s).then_inc(st_sem, 16)
            sync.dma_start(chunk(3), t[:, :]).then_inc(st_sem, 16)
            sync.dma_start(tail, t[64:65, : TAIL_ROWS * EMBED_DIM]).then_inc(st_sem, 16)
            sync.wait_ge(st_sem, 16 * N_STORES)

        @blk.scalar
        def _(scalar):
            for i in range(NPIECE):
                scalar.wait_ge(ld_sems[i], 16)
                scalar.dma_start(
                    c0[:, i * PW : (i + 1) * PW], t[:, i * PW : (i + 1) * PW]
                ).then_inc(st_sem, 16)
            scalar.dma_start(chunk(2), t[:, :]).then_inc(st_sem, 16)
            for j in (1, 3):
                d, s = rem_strip(j)
                scalar.dma_start(d, s).then_inc(st_sem, 16)
            scalar.dma_start(chunk(4), t[:, :]).then_inc(st_sem, 16)
    nc.compile()
    return nc


def _get_program():
    if "nc" not in _CACHE:
        _CACHE["nc"] = _build_program()
    return _CACHE["nc"]


def _host_blocks(price_w, size_w, exchange_w, pair_w):
    """Per-core [128, SUPER] f32 pattern blocks (all partitions identical)."""
    idx = np.arange(PERIOD)
    row12 = np.concatenate(
        [
            np.broadcast_to(price_w[0], (PERIOD, D4)),
            np.broadcast_to(size_w[0], (PERIOD, D4)),
            exchange_w[idx % 3],
            pair_w[idx % 4],
        ],
        axis=-1,
    ).astype(np.float32)  # [12, 512]
    blocks = []
    for c in range(N_CORES):
        base = c * ROWS_PER_CORE
        s = row12[(base + idx) % PERIOD].reshape(-1)  # [SUPER]
        blocks.append(np.ascontiguousarray(np.broadcast_to(s, (128, SUPER))))
    return blocks


def kernel(num_features, price_w, size_w, exchange_w, pair_w):
    global LAST_EXEC_NS, LAST_RESULTS
    from concourse.bass_utils import run_bass_kernel_spmd

    assert int(num_features) == NUM_FEATURES
    price_w = np.asarray(price_w, dtype=np.float32)
    size_w = np.asarray(size_w, dtype=np.float32)
    exchange_w = np.asarray(exchange_w, dtype=np.float32)
    pair_w = np.asarray(pair_w, dtype=np.float32)

    nc = _get_program()
    in_maps = [{"block": b} for b in _host_blocks(price_w, size_w, exchange_w, pair_w)]
    res = run_bass_kernel_spmd(nc, in_maps, list(range(N_CORES)), trace=TRACE)
    LAST_EXEC_NS = res.exec_time_ns
    LAST_RESULTS = res
    return np.concatenate([res.results[c]["out"] for c in range(N_CORES)], axis=0)
